# revision 37
# baseline (speedup 1.0000x reference)
"""Multi-head causal attention (B=2, T=2048, E=1024, H=16, D=64) on 8 trn2 cores.

Sharding: core c -> batch b = c // 4, head-group hg = c % 4 (4 heads each).
Per-core: QKV projections for its 4 heads, causal flash attention in
transposed-score layout (S^T[k,q]; softmax denominator folded into a
ones-augmented V matmul), row-parallel output projection producing a partial
[T, E] output. Host sums the 4 partials per batch and adds the bias.
"""
import sys
from contextlib import ExitStack

sys.path.insert(0, "/opt/trn_rl_repo")

import numpy as np

import concourse.bass as bass
import concourse.tile as tile
from concourse import bacc, mybir
from concourse.bass_utils import run_bass_kernel_spmd

F32 = mybir.dt.float32
F32R = mybir.dt.float32r
EXP = mybir.ActivationFunctionType.Exp

B, T, E, H = 2, 2048, 1024, 16
D = E // H              # 64
N_CORES = 8
GH = 4                  # heads per core
GE = GH * D             # 256 per-core projection width
SCALE = float(D) ** -0.5

TCH = 512               # projection t-chunk
NTCH = T // TCH         # 4
KC = 8                  # contraction chunks of 128 over E
QB = 512                # attention q-block
NQB = T // QB           # 4
KB = 128                # attention k-block


DEFAULT_OPTS = dict(
    interleave=False,    # head-interleaved emission (PE row-group packing) -- off: modeled slower
    s_bufs=2,            # S psum slots ([128,1024] = 2 banks each)
    y_in_s=False,
    o_bufs=3,
    p_bufs=6,
    x_bufs=10,
    v_bufs=3,
    y_split=True,        # Y psum as two single-bank [128,512] tiles
    y_bufs=1,
    y_last_in_s=True,    # final q-block Y tiles borrow the idle S slots
    recip_fast=False,    # approx recip (custom DVE) produced garbage on HW -- keep exact
    norm_splits_last=1,  # split normalize unverified on HW -- keep single-chain
)


def build_program(opts=None):
    o = dict(DEFAULT_OPTS)
    if opts:
        o.update(opts)
    nc = bacc.Bacc("TRN2", target_bir_lowering=False, debug=False, num_devices=N_CORES)

    xt_d = nc.dram_tensor("xt", [E, T], F32, kind="ExternalInput").ap()
    wqt_d = nc.dram_tensor("wqt", [E, GE], F32, kind="ExternalInput").ap()
    wkt_d = nc.dram_tensor("wkt", [E, GE], F32, kind="ExternalInput").ap()
    wvt_d = nc.dram_tensor("wvt", [E, GE], F32, kind="ExternalInput").ap()
    wpt_d = nc.dram_tensor("wpt", [GE, E], F32, kind="ExternalInput").ap()
    tri_d = nc.dram_tensor("tri", [KB, KB], F32, kind="ExternalInput").ap()
    ones_d = nc.dram_tensor("ones", [128, (T // KB) * GH], F32, kind="ExternalInput").ap()
    y_d = nc.dram_tensor("y", [T, E], F32, kind="ExternalOutput").ap()

    with tile.TileContext(nc) as tc:
        with tc.tile_pool(name="weights", bufs=1) as wpool, \
             tc.tile_pool(name="qk", bufs=1) as qkpool, \
             tc.tile_pool(name="vsb", bufs=1) as vpool, \
             tc.tile_pool(name="xin", bufs=o["x_bufs"]) as xpool, \
             tc.tile_pool(name="ptile", bufs=o["p_bufs"]) as ppool, \
             tc.tile_pool(name="osb", bufs=3) as opool, \
             tc.tile_pool(name="lbc", bufs=3) as lpool, \
             tc.tile_pool(name="onorm", bufs=4) as onpool, \
             tc.tile_pool(name="ystage", bufs=2) as ypool:

            # --- weights / mask tiles (DMAs emitted inside phase 1 so x
            # transfers come first and matmuls start early) ---
            wq_sb = wpool.tile([128, KC, GE], F32R)
            wk_sb = wpool.tile([128, KC, GE], F32R)
            wv_sb = wpool.tile([128, KC, GE], F32R)
            wp_sb = wpool.tile([128, 2, E], F32R)
            tri_sb = wpool.tile([KB, KB], F32R)

            def load_weight_chunk(kc, which):
                for w_sb, w_d in which:
                    nc.sync.dma_start(
                        out=w_sb[:, kc, :],
                        in_=w_d.bitcast(F32R)[kc * 128:(kc + 1) * 128, :])

            qt_sb = qkpool.tile([128, 2, T], F32R)   # pair-stacked Q^T
            kt_sb = qkpool.tile([128, 2, T], F32R)   # pair-stacked K^T
            v_sb = vpool.tile([128, T // KB, GH * (D + 1)], F32R)  # [k-part, kblock, head-slot(64 V + 1 ones)]

            # ones columns of the augmented V (col D of each 65-wide head
            # slot); loaded after the first x chunks to keep startup clean
            v_ones = v_sb.rearrange("p b (h c) -> p (b h) c", c=D + 1)[:, :, D:D + 1]
            ones_sb = wpool.tile([128, (T // KB) * GH], F32R)
            nc.sync.dma_start(out=ones_sb[:], in_=ones_d.bitcast(F32R))
            nc.vector.tensor_copy(
                out=v_ones,
                in_=ones_sb.rearrange("p (n o) -> p n o", o=1),
            )

            # --- phase 1: projections ---
            proj_ctx = ExitStack()
            qk_ps = proj_ctx.enter_context(tc.tile_pool(name="qk_ps", bufs=o.get("qk_bufs", 2), space="PSUM"))
            v_ps = proj_ctx.enter_context(tc.tile_pool(name="v_ps", bufs=o.get("v_bufs", 2), space="PSUM"))
            for tch in range(NTCH):
                ts0 = tch * TCH
                xts = []
                for kc in range(KC):
                    xt = xpool.tile([128, TCH], F32R, tag="xt")
                    nc.sync.dma_start(out=xt[:], in_=xt_d.bitcast(F32R)[kc * 128:(kc + 1) * 128, ts0:ts0 + TCH])
                    xts.append(xt)
                    if tch == 0:
                        # q/k weights ride along with their x chunk; v weights
                        # (used later in the t-chunk) trail by 4 chunks
                        load_weight_chunk(kc, ((wq_sb, wqt_d), (wk_sb, wkt_d)))
                        if kc >= 4:
                            load_weight_chunk(kc - 4, ((wv_sb, wvt_d),))
                if tch == 0:
                    for kc in range(4, KC):
                        load_weight_chunk(kc, ((wv_sb, wvt_d),))
                    nc.sync.dma_start(out=tri_sb[:], in_=tri_d.bitcast(F32R))
                    nc.sync.dma_start(out=wp_sb[:], in_=wpt_d.bitcast(F32R).rearrange("(c p) n -> p c n", p=128))
                for pair in range(2):
                    psl = slice(pair * 128, (pair + 1) * 128)
                    qp = qk_ps.tile([128, TCH], F32)
                    for kc in range(KC):
                        nc.tensor.matmul(qp[:], wq_sb[:, kc, psl], xts[kc][:],
                                         start=(kc == 0), stop=(kc == KC - 1))
                    nc.vector.tensor_copy(out=qt_sb[:, pair, ts0:ts0 + TCH], in_=qp[:])
                    kp = qk_ps.tile([128, TCH], F32)
                    for kc in range(KC):
                        nc.tensor.matmul(kp[:], wk_sb[:, kc, psl], xts[kc][:],
                                         start=(kc == 0), stop=(kc == KC - 1))
                    nc.vector.tensor_copy(out=kt_sb[:, pair, ts0:ts0 + TCH], in_=kp[:])
                for tsub in range(TCH // KB):
                    vp = v_ps.tile([128, GE], F32)
                    for kc in range(KC):
                        nc.tensor.matmul(vp[:], xts[kc][:, tsub * KB:(tsub + 1) * KB], wv_sb[:, kc, :],
                                         start=(kc == 0), stop=(kc == KC - 1))
                    tb = tch * (TCH // KB) + tsub
                    nc.vector.tensor_copy(
                        out=v_sb[:, tb, :].rearrange("p (h c) -> p h c", c=D + 1)[:, :, 0:D],
                        in_=vp.rearrange("p (h c) -> p h c", c=D),
                    )

            proj_ctx.close()

            # --- phase 2+3: attention per q-block, then its output projection ---
            attn_ctx = ExitStack()
            s_ps = attn_ctx.enter_context(tc.tile_pool(name="s_ps", bufs=o["s_bufs"], space="PSUM"))
            o_ps = attn_ctx.enter_context(tc.tile_pool(name="o_ps", bufs=o["o_bufs"], space="PSUM"))
            if o["y_in_s"]:
                y_ps, y_tag = s_ps, "s"
            elif o.get("y_in_o", False):
                y_ps, y_tag = o_ps, "o"
            else:
                y_ps = attn_ctx.enter_context(
                    tc.tile_pool(name="y_ps", bufs=o.get("y_bufs", 1), space="PSUM"))
                y_tag = "y"

            def slot(hb):
                return slice(hb * (D + 1), (hb + 1) * (D + 1))

            def tri_eng(pt, r):
                eng = nc.gpsimd if o.get("tri_gpsimd", False) else nc.vector
                eng.tensor_mul(pt[:, r:r + KB], pt[:, r:r + KB], tri_sb[:])

            def normalize(o_p, onorm, h, splits=1):
                # reciprocal of the l row (PSUM partition 64 -> SBUF partition
                # 0; DVE handles the base shift), gpsimd-broadcast across 64
                # partitions, then normalize straight out of PSUM into the
                # pair-stacked onorm tile (head B writes partitions 64:128).
                # splits>1 chops the chain along q so downstream Y matmuls
                # start on the first q-tile sooner (used for the last q-block).
                w = QB // splits
                for s in range(splits):
                    qs = slice(s * w, (s + 1) * w)
                    strip = lpool.tile([1, w], F32, tag="strip")
                    if o.get("recip_fast", True):
                        nc.vector.reciprocal_approx_fast(out=strip[:], in_=o_p[D:D + 1, qs])
                    else:
                        nc.vector.reciprocal(out=strip[:], in_=o_p[D:D + 1, qs])
                    lb = lpool.tile([D, w], F32, tag="lb")
                    nc.gpsimd.partition_broadcast(lb[:], strip[:])
                    nc.vector.tensor_mul(onorm[h * D:(h + 1) * D, qs], o_p[0:D, qs], lb[:])

            for qb in o.get("qb_order", list(range(NQB))):
                q0 = qb * QB
                nk = (q0 + QB) // KB          # kblocks 0..nk-1; last 4 are diagonal
                nfull = nk - 4
                onorms = []
                for pair in range(2):
                    onorm = onpool.tile([128, QB], F32R)
                    heads = [0, 1] if o["interleave"] else None
                    if o["interleave"]:
                        o_ps_t = [o_ps.tile([D + 1, QB], F32, tag="o", name="o_t") for _ in range(2)]
                        qr = [qt_sb[h * D:(h + 1) * D, pair, q0:q0 + QB] for h in range(2)]
                        # full k-blocks two at a time; S matmuls for the two
                        # heads adjacent (disjoint PE row groups -> HW overlap)
                        for j2 in range(0, nfull, 2):
                            sps = [s_ps.tile([128, 2 * QB], F32, tag="s", name="s_t") for _ in range(2)]
                            for jj in range(2):
                                j = j2 + jj
                                for h in range(2):
                                    nc.tensor.matmul(
                                        sps[h][:, jj * QB:(jj + 1) * QB],
                                        kt_sb[h * D:(h + 1) * D, pair, j * KB:(j + 1) * KB],
                                        qr[h], start=True, stop=True)
                            pts = []
                            for h in range(2):
                                pt = ppool.tile([128, 2 * QB], F32R, tag="p")
                                nc.scalar.activation(out=pt[:], in_=sps[h][:], func=EXP, scale=SCALE)
                                pts.append(pt)
                            for jj in range(2):
                                j = j2 + jj
                                for h in range(2):
                                    nc.tensor.matmul(
                                        o_ps_t[h][:], v_sb[:, j, slot(pair * 2 + h)],
                                        pts[h][:, jj * QB:(jj + 1) * QB],
                                        start=(j == 0), stop=False)
                        for j in range(nfull, nk):
                            r = (j - nfull) * KB
                            sps = [s_ps.tile([128, 2 * QB], F32, tag="s", name="s_t") for _ in range(2)]
                            for h in range(2):
                                nc.tensor.matmul(
                                    sps[h][:, 0:QB],
                                    kt_sb[h * D:(h + 1) * D, pair, j * KB:(j + 1) * KB],
                                    qr[h], start=True, stop=True)
                            for h in range(2):
                                pt = ppool.tile([128, 2 * QB], F32R, tag="p")
                                nc.scalar.activation(out=pt[:, r:QB], in_=sps[h][:, r:QB], func=EXP, scale=SCALE)
                                tri_eng(pt, r)
                                nc.tensor.matmul(
                                    o_ps_t[h][:, r:QB], v_sb[:, j, slot(pair * 2 + h)],
                                    pt[:, r:QB], start=(j == 0), stop=(j == nk - 1))
                        for h in range(2):
                            normalize(o_ps_t[h], onorm, h)
                    else:
                        for h in range(2):
                            hb = pair * 2 + h
                            bsl = slice(h * D, h * D + D)
                            o_p = o_ps.tile([D + 1, QB], F32)
                            qrhs = qt_sb[bsl, pair, q0:q0 + QB]
                            for j2 in range(0, nfull, 2):
                                sp = s_ps.tile([128, 2 * QB], F32, tag="s")
                                for jj in range(2):
                                    j = j2 + jj
                                    nc.tensor.matmul(sp[:, jj * QB:(jj + 1) * QB],
                                                     kt_sb[bsl, pair, j * KB:(j + 1) * KB],
                                                     qrhs, start=True, stop=True)
                                pt = ppool.tile([128, 2 * QB], F32R, tag="p")
                                nc.scalar.activation(out=pt[:], in_=sp[:], func=EXP, scale=SCALE)
                                for jj in range(2):
                                    j = j2 + jj
                                    nc.tensor.matmul(o_p[:], v_sb[:, j, slot(hb)],
                                                     pt[:, jj * QB:(jj + 1) * QB],
                                                     start=(j == 0), stop=False)
                            for j2 in range(nfull, nk, 2):
                                # two diagonal k-blocks share one 2-bank psum
                                # tile and a single exp over [r0 : QB+r1+KB]
                                # (the [QB : QB+r1) strip is unread garbage)
                                r0 = (j2 - nfull) * KB
                                r1 = r0 + KB
                                sp = s_ps.tile([128, 2 * QB], F32, tag="s")
                                for jj in range(2):
                                    j = j2 + jj
                                    nc.tensor.matmul(sp[:, jj * QB:(jj + 1) * QB],
                                                     kt_sb[bsl, pair, j * KB:(j + 1) * KB],
                                                     qrhs, start=True, stop=True)
                                pt = ppool.tile([128, 2 * QB], F32R, tag="p")
                                nc.scalar.activation(out=pt[:, r0:2 * QB], in_=sp[:, r0:2 * QB],
                                                     func=EXP, scale=SCALE)
                                for jj, r in ((0, r0), (1, r1)):
                                    j = j2 + jj
                                    base_c = jj * QB
                                    nc.vector.tensor_mul(pt[:, base_c + r:base_c + r + KB],
                                                         pt[:, base_c + r:base_c + r + KB], tri_sb[:])
                                    nc.tensor.matmul(o_p[:, r:QB], v_sb[:, j, slot(hb)],
                                                     pt[:, base_c + r:base_c + QB],
                                                     start=(j == 0), stop=(j == nk - 1))
                            normalize(o_p, onorm, h,
                                      splits=(o.get("norm_splits_last", 1) if qb == NQB - 1 else 1))
                    onorms.append(onorm)
                # output projection for this q-block
                for qt in range(QB // 128):
                    if o.get("y_split", False):
                        yt = ypool.tile([128, E], F32)
                        for nh in range(2):
                            if o.get("y_in_o", False):
                                yp = o_ps.tile([128, 512], F32, tag="o" if o["interleave"] else "o_p", name="yp")
                            elif o.get("y_last_in_s", False) and qb == NQB - 1:
                                yp = s_ps.tile([128, 512], F32, tag="s", name="yp")
                            else:
                                yp = y_ps.tile([128, 512], F32, tag=y_tag, name="yp")
                            for pair in range(2):
                                nc.tensor.matmul(yp[:],
                                                 onorms[pair][:, qt * 128:(qt + 1) * 128],
                                                 wp_sb[:, pair, nh * 512:(nh + 1) * 512],
                                                 start=(pair == 0), stop=(pair == 1))
                            if o.get("y_copy_act", False) and nh == 1:
                                nc.scalar.activation(out=yt[:, nh * 512:(nh + 1) * 512], in_=yp[:],
                                                     func=mybir.ActivationFunctionType.Copy)
                            else:
                                nc.vector.tensor_copy(out=yt[:, nh * 512:(nh + 1) * 512], in_=yp[:])
                        nc.sync.dma_start(out=y_d[q0 + qt * 128:q0 + (qt + 1) * 128, :], in_=yt[:])
                    else:
                        yp = y_ps.tile([128, E], F32, tag=y_tag)
                        for nh in range(2):
                            for pair in range(2):
                                nc.tensor.matmul(yp[:, nh * 512:(nh + 1) * 512],
                                                 onorms[pair][:, qt * 128:(qt + 1) * 128],
                                                 wp_sb[:, pair, nh * 512:(nh + 1) * 512],
                                                 start=(pair == 0), stop=(pair == 1))
                        yt = ypool.tile([128, E], F32)
                        nc.vector.tensor_copy(out=yt[:], in_=yp[:])
                        nc.sync.dma_start(out=y_d[q0 + qt * 128:q0 + (qt + 1) * 128, :], in_=yt[:])
            attn_ctx.close()

    nc.compile()
    return nc


_NC = {}


def _get_program(opts=None):
    key = tuple(sorted((opts or {}).items()))
    if key not in _NC:
        _NC[key] = build_program(opts)
    return _NC[key]


def _make_in_maps(x, Wq, Wk, Wv, Wp):
    x = np.asarray(x, dtype=np.float32)
    wqt = np.ascontiguousarray(np.asarray(Wq, np.float32).T)
    wkt = np.ascontiguousarray(np.asarray(Wk, np.float32).T)
    wvt = np.ascontiguousarray(np.asarray(Wv, np.float32).T)
    wpt = np.ascontiguousarray(np.asarray(Wp, np.float32).T)
    tri = (np.arange(KB)[:, None] <= np.arange(KB)[None, :]).astype(np.float32)
    ones = np.ones((128, (T // KB) * GH), np.float32)
    in_maps = []
    for c in range(N_CORES):
        b, hg = c // 4, c % 4
        in_maps.append({
            "xt": np.ascontiguousarray(x[b].T),
            "wqt": np.ascontiguousarray(wqt[:, hg * GE:(hg + 1) * GE]),
            "wkt": np.ascontiguousarray(wkt[:, hg * GE:(hg + 1) * GE]),
            "wvt": np.ascontiguousarray(wvt[:, hg * GE:(hg + 1) * GE]),
            "wpt": np.ascontiguousarray(wpt[hg * GE:(hg + 1) * GE, :]),
            "tri": tri,
            "ones": ones,
        })
    return in_maps


def run_cores(x, Wq, Wk, Wv, Wp, bp, **spmd_kwargs):
    """Run the 8-core program; returns (y_full, BassKernelResults)."""
    nc = _get_program()
    in_maps = _make_in_maps(x, Wq, Wk, Wv, Wp)
    res = run_bass_kernel_spmd(nc, in_maps, list(range(N_CORES)), **spmd_kwargs)
    parts = [res.results[c]["y"] for c in range(N_CORES)]
    y = np.empty((B, T, E), np.float32)
    for b in range(B):
        acc = parts[4 * b].astype(np.float32)
        for hg in range(1, 4):
            acc = acc + parts[4 * b + hg]
        y[b] = acc
    y += np.asarray(bp, np.float32)[None, None, :]
    return y, res


def kernel(x, Wq, Wk, Wv, Wp, bp):
    y, _ = run_cores(x, Wq, Wk, Wv, Wp, bp)
    return y


def bench(x, Wq, Wk, Wv, Wp, bp, iters=12):
    """Time repeated on-device executions of the compiled program.

    Returns (y_full, list_of_call_seconds). Builds the sharded jit once;
    inputs are device-resident; fresh donated zero outputs are staged
    outside the timed region each iteration.
    """
    import time

    import jax
    import numpy as np_
    from jax.experimental.shard_map import shard_map
    from jax.sharding import Mesh, NamedSharding, PartitionSpec

    from concourse import bass2jax, mybir as mb

    nc = _get_program()
    in_maps = _make_in_maps(x, Wq, Wk, Wv, Wp)
    n_cores = N_CORES
    bass2jax.install_neuronx_cc_hook()

    partition_name = nc.partition_id_tensor.name if nc.partition_id_tensor else None
    in_names, out_names, out_avals, zero_outs = [], [], [], []
    for alloc in nc.m.functions[0].allocations:
        if not isinstance(alloc, mb.MemoryLocationSet):
            continue
        name = alloc.memorylocations[0].name
        if alloc.kind == "ExternalInput":
            if name != partition_name:
                in_names.append(name)
        elif alloc.kind == "ExternalOutput":
            out_names.append(name)
            shape = tuple(alloc.tensor_shape)
            dtype = mb.dt.np(alloc.dtype)
            out_avals.append(jax.core.ShapedArray(shape, dtype))
            zero_outs.append(np_.zeros(shape, dtype))
    n_params = len(in_names)
    all_in_names = in_names + out_names
    if partition_name is not None:
        all_in_names = all_in_names + [partition_name]

    def _body(*args):
        operands = list(args)
        if partition_name is not None:
            operands.append(bass2jax.partition_id_tensor())
        outs = bass2jax._bass_exec_p.bind(
            *operands,
            out_avals=tuple(out_avals),
            in_names=tuple(all_in_names),
            out_names=tuple(out_names),
            lowering_input_output_aliases=(),
            sim_require_finite=True,
            sim_require_nnan=True,
            nc=nc,
        )
        return tuple(outs)

    devices = jax.devices()[:n_cores]
    mesh = Mesh(np_.asarray(devices), ("core",))
    donate = tuple(range(n_params, n_params + len(out_names)))
    sharded = jax.jit(
        shard_map(_body, mesh=mesh,
                  in_specs=(PartitionSpec("core"),) * (n_params + len(out_names)),
                  out_specs=(PartitionSpec("core"),) * len(out_names),
                  check_rep=False),
        donate_argnums=donate, keep_unused=True,
    )
    sh = NamedSharding(mesh, PartitionSpec("core"))
    concat_in = [
        jax.device_put(
            np_.concatenate([np_.asarray(in_maps[c][nm]) for c in range(n_cores)], axis=0), sh)
        for nm in in_names
    ]
    zeros_np = [np_.zeros((n_cores * z.shape[0], *z.shape[1:]), z.dtype) for z in zero_outs]

    times = []
    out_arrs = None
    for it in range(iters):
        dz = [jax.device_put(z, sh) for z in zeros_np]
        jax.block_until_ready(dz)
        t0 = time.perf_counter()
        out_arrs = sharded(*concat_in, *dz)
        jax.block_until_ready(out_arrs)
        times.append(time.perf_counter() - t0)

    parts = [
        np_.asarray(out_arrs[i]).reshape(n_cores, *out_avals[i].shape)
        for i, nm in enumerate(out_names)
    ]
    yi = out_names.index("y")
    y = np_.empty((B, T, E), np_.float32)
    for b in range(B):
        acc = parts[yi][4 * b].astype(np_.float32)
        for hg in range(1, 4):
            acc = acc + parts[yi][4 * b + hg]
        y[b] = acc
    y += np_.asarray(bp, np_.float32)[None, None, :]
    return y, times


# revision 39
# speedup vs baseline: 1.0041x; 1.0041x over previous
"""Multi-head causal attention (B=2, T=2048, E=1024, H=16, D=64) on 8 trn2 cores.

Sharding: core c -> batch b = c // 4, head-group hg = c % 4 (4 heads each).
Per-core: QKV projections for its 4 heads, causal flash attention in
transposed-score layout (S^T[k,q]; softmax denominator folded into a
ones-augmented V matmul), row-parallel output projection producing a partial
[T, E] output. Host sums the 4 partials per batch and adds the bias.
"""
import sys
from contextlib import ExitStack

sys.path.insert(0, "/opt/trn_rl_repo")

import numpy as np

import concourse.bass as bass
import concourse.tile as tile
from concourse import bacc, mybir
from concourse.bass_utils import run_bass_kernel_spmd

F32 = mybir.dt.float32
F32R = mybir.dt.float32r
EXP = mybir.ActivationFunctionType.Exp

B, T, E, H = 2, 2048, 1024, 16
D = E // H              # 64
N_CORES = 8
GH = 4                  # heads per core
GE = GH * D             # 256 per-core projection width
SCALE = float(D) ** -0.5

TCH = 512               # projection t-chunk
NTCH = T // TCH         # 4
KC = 8                  # contraction chunks of 128 over E
QB = 512                # attention q-block
NQB = T // QB           # 4
KB = 128                # attention k-block


DEFAULT_OPTS = dict(
    interleave=False,    # head-interleaved emission (PE row-group packing) -- off: modeled slower
    s_bufs=2,            # S psum slots ([128,1024] = 2 banks each)
    y_in_s=False,
    o_bufs=3,
    p_bufs=6,
    x_bufs=10,
    v_bufs=3,
    y_split=True,        # Y psum as two single-bank [128,512] tiles
    y_bufs=1,
    y_last_in_s=True,    # final q-block Y tiles borrow the idle S slots
    recip_fast=False,    # approx recip (custom DVE) produced garbage on HW -- keep exact
    norm_splits_last=4,  # split the last q-block's normalize per q-tile
)


def build_program(opts=None):
    o = dict(DEFAULT_OPTS)
    if opts:
        o.update(opts)
    nc = bacc.Bacc("TRN2", target_bir_lowering=False, debug=False, num_devices=N_CORES)

    xt_d = nc.dram_tensor("xt", [E, T], F32, kind="ExternalInput").ap()
    wqt_d = nc.dram_tensor("wqt", [E, GE], F32, kind="ExternalInput").ap()
    wkt_d = nc.dram_tensor("wkt", [E, GE], F32, kind="ExternalInput").ap()
    wvt_d = nc.dram_tensor("wvt", [E, GE], F32, kind="ExternalInput").ap()
    wpt_d = nc.dram_tensor("wpt", [GE, E], F32, kind="ExternalInput").ap()
    tri_d = nc.dram_tensor("tri", [KB, KB], F32, kind="ExternalInput").ap()
    ones_d = nc.dram_tensor("ones", [128, (T // KB) * GH], F32, kind="ExternalInput").ap()
    y_d = nc.dram_tensor("y", [T, E], F32, kind="ExternalOutput").ap()

    with tile.TileContext(nc) as tc:
        with tc.tile_pool(name="weights", bufs=1) as wpool, \
             tc.tile_pool(name="qk", bufs=1) as qkpool, \
             tc.tile_pool(name="vsb", bufs=1) as vpool, \
             tc.tile_pool(name="xin", bufs=o["x_bufs"]) as xpool, \
             tc.tile_pool(name="ptile", bufs=o["p_bufs"]) as ppool, \
             tc.tile_pool(name="osb", bufs=3) as opool, \
             tc.tile_pool(name="lbc", bufs=3) as lpool, \
             tc.tile_pool(name="onorm", bufs=4) as onpool, \
             tc.tile_pool(name="ystage", bufs=2) as ypool:

            # --- weights / mask tiles (DMAs emitted inside phase 1 so x
            # transfers come first and matmuls start early) ---
            wq_sb = wpool.tile([128, KC, GE], F32R)
            wk_sb = wpool.tile([128, KC, GE], F32R)
            wv_sb = wpool.tile([128, KC, GE], F32R)
            wp_sb = wpool.tile([128, 2, E], F32R)
            tri_sb = wpool.tile([KB, KB], F32R)

            def load_weight_chunk(kc, which):
                for w_sb, w_d in which:
                    nc.sync.dma_start(
                        out=w_sb[:, kc, :],
                        in_=w_d.bitcast(F32R)[kc * 128:(kc + 1) * 128, :])

            qt_sb = qkpool.tile([128, 2, T], F32R)   # pair-stacked Q^T
            kt_sb = qkpool.tile([128, 2, T], F32R)   # pair-stacked K^T
            v_sb = vpool.tile([128, T // KB, GH * (D + 1)], F32R)  # [k-part, kblock, head-slot(64 V + 1 ones)]

            # ones columns of the augmented V (col D of each 65-wide head
            # slot); loaded after the first x chunks to keep startup clean
            v_ones = v_sb.rearrange("p b (h c) -> p (b h) c", c=D + 1)[:, :, D:D + 1]
            ones_sb = wpool.tile([128, (T // KB) * GH], F32R)
            nc.sync.dma_start(out=ones_sb[:], in_=ones_d.bitcast(F32R))
            nc.vector.tensor_copy(
                out=v_ones,
                in_=ones_sb.rearrange("p (n o) -> p n o", o=1),
            )

            # --- phase 1: projections ---
            proj_ctx = ExitStack()
            qk_ps = proj_ctx.enter_context(tc.tile_pool(name="qk_ps", bufs=o.get("qk_bufs", 2), space="PSUM"))
            v_ps = proj_ctx.enter_context(tc.tile_pool(name="v_ps", bufs=o.get("v_bufs", 2), space="PSUM"))
            for tch in range(NTCH):
                ts0 = tch * TCH
                xts = []
                split0 = o.get("x_split_first", False) and tch == 0
                for kc in range(KC):
                    xt = xpool.tile([128, TCH], F32R, tag="xt")
                    if split0:
                        # halved transfers so the first matmuls start sooner
                        for hf in range(2):
                            nc.sync.dma_start(
                                out=xt[:, hf * (TCH // 2):(hf + 1) * (TCH // 2)],
                                in_=xt_d.bitcast(F32R)[kc * 128:(kc + 1) * 128,
                                                       ts0 + hf * (TCH // 2):ts0 + (hf + 1) * (TCH // 2)])
                    else:
                        nc.sync.dma_start(out=xt[:], in_=xt_d.bitcast(F32R)[kc * 128:(kc + 1) * 128, ts0:ts0 + TCH])
                    xts.append(xt)
                    if tch == 0:
                        # q/k weights ride along with their x chunk; v weights
                        # (used later in the t-chunk) trail by 4 chunks
                        load_weight_chunk(kc, ((wq_sb, wqt_d), (wk_sb, wkt_d)))
                        if kc >= 4:
                            load_weight_chunk(kc - 4, ((wv_sb, wvt_d),))
                if tch == 0:
                    for kc in range(4, KC):
                        load_weight_chunk(kc, ((wv_sb, wvt_d),))
                    nc.sync.dma_start(out=tri_sb[:], in_=tri_d.bitcast(F32R))
                    nc.sync.dma_start(out=wp_sb[:], in_=wpt_d.bitcast(F32R).rearrange("(c p) n -> p c n", p=128))
                halves = ((0, TCH // 2), (TCH // 2, TCH)) if split0 else ((0, TCH),)
                for pair in range(2):
                    psl = slice(pair * 128, (pair + 1) * 128)
                    qp = qk_ps.tile([128, TCH], F32)
                    for h0, h1 in halves:
                        for kc in range(KC):
                            nc.tensor.matmul(qp[:, h0:h1], wq_sb[:, kc, psl], xts[kc][:, h0:h1],
                                             start=(kc == 0), stop=(kc == KC - 1))
                    nc.vector.tensor_copy(out=qt_sb[:, pair, ts0:ts0 + TCH], in_=qp[:])
                    kp = qk_ps.tile([128, TCH], F32)
                    for h0, h1 in halves:
                        for kc in range(KC):
                            nc.tensor.matmul(kp[:, h0:h1], wk_sb[:, kc, psl], xts[kc][:, h0:h1],
                                             start=(kc == 0), stop=(kc == KC - 1))
                    nc.vector.tensor_copy(out=kt_sb[:, pair, ts0:ts0 + TCH], in_=kp[:])
                for tsub in range(TCH // KB):
                    vp = v_ps.tile([128, GE], F32)
                    for kc in range(KC):
                        nc.tensor.matmul(vp[:], xts[kc][:, tsub * KB:(tsub + 1) * KB], wv_sb[:, kc, :],
                                         start=(kc == 0), stop=(kc == KC - 1))
                    tb = tch * (TCH // KB) + tsub
                    nc.vector.tensor_copy(
                        out=v_sb[:, tb, :].rearrange("p (h c) -> p h c", c=D + 1)[:, :, 0:D],
                        in_=vp.rearrange("p (h c) -> p h c", c=D),
                    )

            proj_ctx.close()

            # --- phase 2+3: attention per q-block, then its output projection ---
            attn_ctx = ExitStack()
            s_ps = attn_ctx.enter_context(tc.tile_pool(name="s_ps", bufs=o["s_bufs"], space="PSUM"))
            o_ps = attn_ctx.enter_context(tc.tile_pool(name="o_ps", bufs=o["o_bufs"], space="PSUM"))
            if o["y_in_s"]:
                y_ps, y_tag = s_ps, "s"
            elif o.get("y_in_o", False):
                y_ps, y_tag = o_ps, "o"
            else:
                y_ps = attn_ctx.enter_context(
                    tc.tile_pool(name="y_ps", bufs=o.get("y_bufs", 1), space="PSUM"))
                y_tag = "y"

            def slot(hb):
                return slice(hb * (D + 1), (hb + 1) * (D + 1))

            def tri_eng(pt, r):
                eng = nc.gpsimd if o.get("tri_gpsimd", False) else nc.vector
                eng.tensor_mul(pt[:, r:r + KB], pt[:, r:r + KB], tri_sb[:])

            def normalize(o_p, onorm, h, splits=1):
                # reciprocal of the l row (PSUM partition 64 -> SBUF partition
                # 0; DVE handles the base shift), gpsimd-broadcast across 64
                # partitions, then normalize straight out of PSUM into the
                # pair-stacked onorm tile (head B writes partitions 64:128).
                # splits>1 chops the chain along q so downstream Y matmuls
                # start on the first q-tile sooner (used for the last q-block).
                w = QB // splits
                for s in range(splits):
                    qs = slice(s * w, (s + 1) * w)
                    strip = lpool.tile([1, w], F32, tag="strip")
                    if o.get("recip_fast", True):
                        nc.vector.reciprocal_approx_fast(out=strip[:], in_=o_p[D:D + 1, qs])
                    else:
                        nc.vector.reciprocal(out=strip[:], in_=o_p[D:D + 1, qs])
                    lb = lpool.tile([D, w], F32, tag="lb")
                    nc.gpsimd.partition_broadcast(lb[:], strip[:])
                    nc.vector.tensor_mul(onorm[h * D:(h + 1) * D, qs], o_p[0:D, qs], lb[:])

            for qb in o.get("qb_order", list(range(NQB))):
                q0 = qb * QB
                nk = (q0 + QB) // KB          # kblocks 0..nk-1; last 4 are diagonal
                nfull = nk - 4
                onorms = []
                for pair in range(2):
                    onorm = onpool.tile([128, QB], F32R)
                    heads = [0, 1] if o["interleave"] else None
                    if o["interleave"]:
                        o_ps_t = [o_ps.tile([D + 1, QB], F32, tag="o", name="o_t") for _ in range(2)]
                        qr = [qt_sb[h * D:(h + 1) * D, pair, q0:q0 + QB] for h in range(2)]
                        # full k-blocks two at a time; S matmuls for the two
                        # heads adjacent (disjoint PE row groups -> HW overlap)
                        for j2 in range(0, nfull, 2):
                            sps = [s_ps.tile([128, 2 * QB], F32, tag="s", name="s_t") for _ in range(2)]
                            for jj in range(2):
                                j = j2 + jj
                                for h in range(2):
                                    nc.tensor.matmul(
                                        sps[h][:, jj * QB:(jj + 1) * QB],
                                        kt_sb[h * D:(h + 1) * D, pair, j * KB:(j + 1) * KB],
                                        qr[h], start=True, stop=True)
                            pts = []
                            for h in range(2):
                                pt = ppool.tile([128, 2 * QB], F32R, tag="p")
                                nc.scalar.activation(out=pt[:], in_=sps[h][:], func=EXP, scale=SCALE)
                                pts.append(pt)
                            for jj in range(2):
                                j = j2 + jj
                                for h in range(2):
                                    nc.tensor.matmul(
                                        o_ps_t[h][:], v_sb[:, j, slot(pair * 2 + h)],
                                        pts[h][:, jj * QB:(jj + 1) * QB],
                                        start=(j == 0), stop=False)
                        for j in range(nfull, nk):
                            r = (j - nfull) * KB
                            sps = [s_ps.tile([128, 2 * QB], F32, tag="s", name="s_t") for _ in range(2)]
                            for h in range(2):
                                nc.tensor.matmul(
                                    sps[h][:, 0:QB],
                                    kt_sb[h * D:(h + 1) * D, pair, j * KB:(j + 1) * KB],
                                    qr[h], start=True, stop=True)
                            for h in range(2):
                                pt = ppool.tile([128, 2 * QB], F32R, tag="p")
                                nc.scalar.activation(out=pt[:, r:QB], in_=sps[h][:, r:QB], func=EXP, scale=SCALE)
                                tri_eng(pt, r)
                                nc.tensor.matmul(
                                    o_ps_t[h][:, r:QB], v_sb[:, j, slot(pair * 2 + h)],
                                    pt[:, r:QB], start=(j == 0), stop=(j == nk - 1))
                        for h in range(2):
                            normalize(o_ps_t[h], onorm, h)
                    else:
                        for h in range(2):
                            hb = pair * 2 + h
                            bsl = slice(h * D, h * D + D)
                            o_p = o_ps.tile([D + 1, QB], F32)
                            qrhs = qt_sb[bsl, pair, q0:q0 + QB]
                            for j2 in range(0, nfull, 2):
                                sp = s_ps.tile([128, 2 * QB], F32, tag="s")
                                for jj in range(2):
                                    j = j2 + jj
                                    nc.tensor.matmul(sp[:, jj * QB:(jj + 1) * QB],
                                                     kt_sb[bsl, pair, j * KB:(j + 1) * KB],
                                                     qrhs, start=True, stop=True)
                                pt = ppool.tile([128, 2 * QB], F32R, tag="p")
                                nc.scalar.activation(out=pt[:], in_=sp[:], func=EXP, scale=SCALE)
                                for jj in range(2):
                                    j = j2 + jj
                                    nc.tensor.matmul(o_p[:], v_sb[:, j, slot(hb)],
                                                     pt[:, jj * QB:(jj + 1) * QB],
                                                     start=(j == 0), stop=False)
                            for j2 in range(nfull, nk, 2):
                                # two diagonal k-blocks share one 2-bank psum
                                # tile and a single exp over [r0 : QB+r1+KB]
                                # (the [QB : QB+r1) strip is unread garbage)
                                r0 = (j2 - nfull) * KB
                                r1 = r0 + KB
                                sp = s_ps.tile([128, 2 * QB], F32, tag="s")
                                for jj in range(2):
                                    j = j2 + jj
                                    nc.tensor.matmul(sp[:, jj * QB:(jj + 1) * QB],
                                                     kt_sb[bsl, pair, j * KB:(j + 1) * KB],
                                                     qrhs, start=True, stop=True)
                                pt = ppool.tile([128, 2 * QB], F32R, tag="p")
                                nc.scalar.activation(out=pt[:, r0:2 * QB], in_=sp[:, r0:2 * QB],
                                                     func=EXP, scale=SCALE)
                                for jj, r in ((0, r0), (1, r1)):
                                    j = j2 + jj
                                    base_c = jj * QB
                                    nc.vector.tensor_mul(pt[:, base_c + r:base_c + r + KB],
                                                         pt[:, base_c + r:base_c + r + KB], tri_sb[:])
                                    nc.tensor.matmul(o_p[:, r:QB], v_sb[:, j, slot(hb)],
                                                     pt[:, base_c + r:base_c + QB],
                                                     start=(j == 0), stop=(j == nk - 1))
                            normalize(o_p, onorm, h,
                                      splits=(o.get("norm_splits_last", 1) if qb == NQB - 1 else 1))
                    onorms.append(onorm)
                # output projection for this q-block
                for qt in range(QB // 128):
                    if o.get("y_split", False):
                        yt = ypool.tile([128, E], F32)
                        for nh in range(2):
                            if o.get("y_in_o", False):
                                yp = o_ps.tile([128, 512], F32, tag="o" if o["interleave"] else "o_p", name="yp")
                            elif o.get("y_last_in_s", False) and qb == NQB - 1:
                                yp = s_ps.tile([128, 512], F32, tag="s", name="yp")
                            else:
                                yp = y_ps.tile([128, 512], F32, tag=y_tag, name="yp")
                            for pair in range(2):
                                nc.tensor.matmul(yp[:],
                                                 onorms[pair][:, qt * 128:(qt + 1) * 128],
                                                 wp_sb[:, pair, nh * 512:(nh + 1) * 512],
                                                 start=(pair == 0), stop=(pair == 1))
                            if o.get("y_copy_act", False) and nh == 1:
                                nc.scalar.activation(out=yt[:, nh * 512:(nh + 1) * 512], in_=yp[:],
                                                     func=mybir.ActivationFunctionType.Copy)
                            else:
                                nc.vector.tensor_copy(out=yt[:, nh * 512:(nh + 1) * 512], in_=yp[:])
                        nc.sync.dma_start(out=y_d[q0 + qt * 128:q0 + (qt + 1) * 128, :], in_=yt[:])
                    else:
                        yp = y_ps.tile([128, E], F32, tag=y_tag)
                        for nh in range(2):
                            for pair in range(2):
                                nc.tensor.matmul(yp[:, nh * 512:(nh + 1) * 512],
                                                 onorms[pair][:, qt * 128:(qt + 1) * 128],
                                                 wp_sb[:, pair, nh * 512:(nh + 1) * 512],
                                                 start=(pair == 0), stop=(pair == 1))
                        yt = ypool.tile([128, E], F32)
                        nc.vector.tensor_copy(out=yt[:], in_=yp[:])
                        nc.sync.dma_start(out=y_d[q0 + qt * 128:q0 + (qt + 1) * 128, :], in_=yt[:])
            attn_ctx.close()

    nc.compile()
    return nc


_NC = {}


def _get_program(opts=None):
    key = tuple(sorted((opts or {}).items()))
    if key not in _NC:
        _NC[key] = build_program(opts)
    return _NC[key]


def _make_in_maps(x, Wq, Wk, Wv, Wp):
    x = np.asarray(x, dtype=np.float32)
    wqt = np.ascontiguousarray(np.asarray(Wq, np.float32).T)
    wkt = np.ascontiguousarray(np.asarray(Wk, np.float32).T)
    wvt = np.ascontiguousarray(np.asarray(Wv, np.float32).T)
    wpt = np.ascontiguousarray(np.asarray(Wp, np.float32).T)
    tri = (np.arange(KB)[:, None] <= np.arange(KB)[None, :]).astype(np.float32)
    ones = np.ones((128, (T // KB) * GH), np.float32)
    in_maps = []
    for c in range(N_CORES):
        b, hg = c // 4, c % 4
        in_maps.append({
            "xt": np.ascontiguousarray(x[b].T),
            "wqt": np.ascontiguousarray(wqt[:, hg * GE:(hg + 1) * GE]),
            "wkt": np.ascontiguousarray(wkt[:, hg * GE:(hg + 1) * GE]),
            "wvt": np.ascontiguousarray(wvt[:, hg * GE:(hg + 1) * GE]),
            "wpt": np.ascontiguousarray(wpt[hg * GE:(hg + 1) * GE, :]),
            "tri": tri,
            "ones": ones,
        })
    return in_maps


def run_cores(x, Wq, Wk, Wv, Wp, bp, **spmd_kwargs):
    """Run the 8-core program; returns (y_full, BassKernelResults)."""
    nc = _get_program()
    in_maps = _make_in_maps(x, Wq, Wk, Wv, Wp)
    res = run_bass_kernel_spmd(nc, in_maps, list(range(N_CORES)), **spmd_kwargs)
    parts = [res.results[c]["y"] for c in range(N_CORES)]
    y = np.empty((B, T, E), np.float32)
    for b in range(B):
        acc = parts[4 * b].astype(np.float32)
        for hg in range(1, 4):
            acc = acc + parts[4 * b + hg]
        y[b] = acc
    y += np.asarray(bp, np.float32)[None, None, :]
    return y, res


def kernel(x, Wq, Wk, Wv, Wp, bp):
    y, _ = run_cores(x, Wq, Wk, Wv, Wp, bp)
    return y


def bench(x, Wq, Wk, Wv, Wp, bp, iters=12):
    """Time repeated on-device executions of the compiled program.

    Returns (y_full, list_of_call_seconds). Builds the sharded jit once;
    inputs are device-resident; fresh donated zero outputs are staged
    outside the timed region each iteration.
    """
    import time

    import jax
    import numpy as np_
    from jax.experimental.shard_map import shard_map
    from jax.sharding import Mesh, NamedSharding, PartitionSpec

    from concourse import bass2jax, mybir as mb

    nc = _get_program()
    in_maps = _make_in_maps(x, Wq, Wk, Wv, Wp)
    n_cores = N_CORES
    bass2jax.install_neuronx_cc_hook()

    partition_name = nc.partition_id_tensor.name if nc.partition_id_tensor else None
    in_names, out_names, out_avals, zero_outs = [], [], [], []
    for alloc in nc.m.functions[0].allocations:
        if not isinstance(alloc, mb.MemoryLocationSet):
            continue
        name = alloc.memorylocations[0].name
        if alloc.kind == "ExternalInput":
            if name != partition_name:
                in_names.append(name)
        elif alloc.kind == "ExternalOutput":
            out_names.append(name)
            shape = tuple(alloc.tensor_shape)
            dtype = mb.dt.np(alloc.dtype)
            out_avals.append(jax.core.ShapedArray(shape, dtype))
            zero_outs.append(np_.zeros(shape, dtype))
    n_params = len(in_names)
    all_in_names = in_names + out_names
    if partition_name is not None:
        all_in_names = all_in_names + [partition_name]

    def _body(*args):
        operands = list(args)
        if partition_name is not None:
            operands.append(bass2jax.partition_id_tensor())
        outs = bass2jax._bass_exec_p.bind(
            *operands,
            out_avals=tuple(out_avals),
            in_names=tuple(all_in_names),
            out_names=tuple(out_names),
            lowering_input_output_aliases=(),
            sim_require_finite=True,
            sim_require_nnan=True,
            nc=nc,
        )
        return tuple(outs)

    devices = jax.devices()[:n_cores]
    mesh = Mesh(np_.asarray(devices), ("core",))
    donate = tuple(range(n_params, n_params + len(out_names)))
    sharded = jax.jit(
        shard_map(_body, mesh=mesh,
                  in_specs=(PartitionSpec("core"),) * (n_params + len(out_names)),
                  out_specs=(PartitionSpec("core"),) * len(out_names),
                  check_rep=False),
        donate_argnums=donate, keep_unused=True,
    )
    sh = NamedSharding(mesh, PartitionSpec("core"))
    concat_in = [
        jax.device_put(
            np_.concatenate([np_.asarray(in_maps[c][nm]) for c in range(n_cores)], axis=0), sh)
        for nm in in_names
    ]
    zeros_np = [np_.zeros((n_cores * z.shape[0], *z.shape[1:]), z.dtype) for z in zero_outs]

    times = []
    out_arrs = None
    for it in range(iters):
        dz = [jax.device_put(z, sh) for z in zeros_np]
        jax.block_until_ready(dz)
        t0 = time.perf_counter()
        out_arrs = sharded(*concat_in, *dz)
        jax.block_until_ready(out_arrs)
        times.append(time.perf_counter() - t0)

    parts = [
        np_.asarray(out_arrs[i]).reshape(n_cores, *out_avals[i].shape)
        for i, nm in enumerate(out_names)
    ]
    yi = out_names.index("y")
    y = np_.empty((B, T, E), np_.float32)
    for b in range(B):
        acc = parts[yi][4 * b].astype(np_.float32)
        for hg in range(1, 4):
            acc = acc + parts[yi][4 * b + hg]
        y[b] = acc
    y += np_.asarray(bp, np_.float32)[None, None, :]
    return y, times


# revision 41
# speedup vs baseline: 1.0169x; 1.0127x over previous
"""Multi-head causal attention (B=2, T=2048, E=1024, H=16, D=64) on 8 trn2 cores.

Sharding: core c -> batch b = c // 4, head-group hg = c % 4 (4 heads each).
Per-core: QKV projections for its 4 heads, causal flash attention in
transposed-score layout (S^T[k,q]; softmax denominator folded into a
ones-augmented V matmul), row-parallel output projection producing a partial
[T, E] output. Host sums the 4 partials per batch and adds the bias.
"""
import sys
from contextlib import ExitStack

sys.path.insert(0, "/opt/trn_rl_repo")

import numpy as np

import concourse.bass as bass
import concourse.tile as tile
from concourse import bacc, mybir
from concourse.bass_utils import run_bass_kernel_spmd

F32 = mybir.dt.float32
F32R = mybir.dt.float32r
EXP = mybir.ActivationFunctionType.Exp

B, T, E, H = 2, 2048, 1024, 16
D = E // H              # 64
N_CORES = 8
GH = 4                  # heads per core
GE = GH * D             # 256 per-core projection width
SCALE = float(D) ** -0.5

TCH = 512               # projection t-chunk
NTCH = T // TCH         # 4
KC = 8                  # contraction chunks of 128 over E
QB = 512                # attention q-block
NQB = T // QB           # 4
KB = 128                # attention k-block


DEFAULT_OPTS = dict(
    interleave=False,    # head-interleaved emission (PE row-group packing) -- off: modeled slower
    s_bufs=2,            # S psum slots ([128,1024] = 2 banks each)
    y_in_s=False,
    o_bufs=3,
    p_bufs=6,
    x_bufs=10,
    v_bufs=3,
    y_split=True,        # Y psum as two single-bank [128,512] tiles
    y_bufs=1,
    y_last_in_s=True,    # final q-block Y tiles borrow the idle S slots
    recip_fast=False,    # approx recip (custom DVE) produced garbage on HW -- keep exact
    norm_splits_last=4,  # split the last q-block's normalize per q-tile
    l_bufs=6,
    on_bufs=6,
    yst_bufs=4,          # more Y staging slots pipeline the out-projection tail
)


def build_program(opts=None):
    o = dict(DEFAULT_OPTS)
    if opts:
        o.update(opts)
    nc = bacc.Bacc("TRN2", target_bir_lowering=False, debug=False, num_devices=N_CORES)

    xt_d = nc.dram_tensor("xt", [E, T], F32, kind="ExternalInput").ap()
    wqt_d = nc.dram_tensor("wqt", [E, GE], F32, kind="ExternalInput").ap()
    wkt_d = nc.dram_tensor("wkt", [E, GE], F32, kind="ExternalInput").ap()
    wvt_d = nc.dram_tensor("wvt", [E, GE], F32, kind="ExternalInput").ap()
    wpt_d = nc.dram_tensor("wpt", [GE, E], F32, kind="ExternalInput").ap()
    tri_d = nc.dram_tensor("tri", [KB, KB], F32, kind="ExternalInput").ap()
    ones_d = nc.dram_tensor("ones", [128, (T // KB) * GH], F32, kind="ExternalInput").ap()
    y_d = nc.dram_tensor("y", [T, E], F32, kind="ExternalOutput").ap()

    with tile.TileContext(nc) as tc:
        with tc.tile_pool(name="weights", bufs=1) as wpool, \
             tc.tile_pool(name="qk", bufs=1) as qkpool, \
             tc.tile_pool(name="vsb", bufs=1) as vpool, \
             tc.tile_pool(name="xin", bufs=o["x_bufs"]) as xpool, \
             tc.tile_pool(name="ptile", bufs=o["p_bufs"]) as ppool, \
             tc.tile_pool(name="osb", bufs=3) as opool, \
             tc.tile_pool(name="lbc", bufs=o.get("l_bufs", 3)) as lpool, \
             tc.tile_pool(name="onorm", bufs=o.get("on_bufs", 4)) as onpool, \
             tc.tile_pool(name="ystage", bufs=o.get("yst_bufs", 2)) as ypool:

            # --- weights / mask tiles (DMAs emitted inside phase 1 so x
            # transfers come first and matmuls start early) ---
            wq_sb = wpool.tile([128, KC, GE], F32R)
            wk_sb = wpool.tile([128, KC, GE], F32R)
            wv_sb = wpool.tile([128, KC, GE], F32R)
            wp_sb = wpool.tile([128, 2, E], F32R)
            tri_sb = wpool.tile([KB, KB], F32R)

            def load_weight_chunk(kc, which):
                for w_sb, w_d in which:
                    nc.sync.dma_start(
                        out=w_sb[:, kc, :],
                        in_=w_d.bitcast(F32R)[kc * 128:(kc + 1) * 128, :])

            qt_sb = qkpool.tile([128, 2, T], F32R)   # pair-stacked Q^T
            kt_sb = qkpool.tile([128, 2, T], F32R)   # pair-stacked K^T
            v_sb = vpool.tile([128, T // KB, GH * (D + 1)], F32R)  # [k-part, kblock, head-slot(64 V + 1 ones)]

            # ones columns of the augmented V (col D of each 65-wide head
            # slot); loaded after the first x chunks to keep startup clean
            v_ones = v_sb.rearrange("p b (h c) -> p (b h) c", c=D + 1)[:, :, D:D + 1]
            ones_sb = wpool.tile([128, (T // KB) * GH], F32R)
            nc.sync.dma_start(out=ones_sb[:], in_=ones_d.bitcast(F32R))
            nc.vector.tensor_copy(
                out=v_ones,
                in_=ones_sb.rearrange("p (n o) -> p n o", o=1),
            )

            # --- phase 1: projections ---
            proj_ctx = ExitStack()
            qk_ps = proj_ctx.enter_context(tc.tile_pool(name="qk_ps", bufs=o.get("qk_bufs", 2), space="PSUM"))
            v_ps = proj_ctx.enter_context(tc.tile_pool(name="v_ps", bufs=o.get("v_bufs", 2), space="PSUM"))
            for tch in range(NTCH):
                ts0 = tch * TCH
                xts = []
                split0 = o.get("x_split_first", False) and tch == 0
                for kc in range(KC):
                    xt = xpool.tile([128, TCH], F32R, tag="xt")
                    if split0:
                        # halved transfers so the first matmuls start sooner
                        for hf in range(2):
                            nc.sync.dma_start(
                                out=xt[:, hf * (TCH // 2):(hf + 1) * (TCH // 2)],
                                in_=xt_d.bitcast(F32R)[kc * 128:(kc + 1) * 128,
                                                       ts0 + hf * (TCH // 2):ts0 + (hf + 1) * (TCH // 2)])
                    else:
                        nc.sync.dma_start(out=xt[:], in_=xt_d.bitcast(F32R)[kc * 128:(kc + 1) * 128, ts0:ts0 + TCH])
                    xts.append(xt)
                    if tch == 0:
                        # q/k weights ride along with their x chunk; v weights
                        # (used later in the t-chunk) trail by 4 chunks
                        load_weight_chunk(kc, ((wq_sb, wqt_d), (wk_sb, wkt_d)))
                        if kc >= 4:
                            load_weight_chunk(kc - 4, ((wv_sb, wvt_d),))
                if tch == 0:
                    for kc in range(4, KC):
                        load_weight_chunk(kc, ((wv_sb, wvt_d),))
                    nc.sync.dma_start(out=tri_sb[:], in_=tri_d.bitcast(F32R))
                    nc.sync.dma_start(out=wp_sb[:], in_=wpt_d.bitcast(F32R).rearrange("(c p) n -> p c n", p=128))
                halves = ((0, TCH // 2), (TCH // 2, TCH)) if split0 else ((0, TCH),)
                for pair in range(2):
                    psl = slice(pair * 128, (pair + 1) * 128)
                    qp = qk_ps.tile([128, TCH], F32)
                    for h0, h1 in halves:
                        for kc in range(KC):
                            nc.tensor.matmul(qp[:, h0:h1], wq_sb[:, kc, psl], xts[kc][:, h0:h1],
                                             start=(kc == 0), stop=(kc == KC - 1))
                    nc.vector.tensor_copy(out=qt_sb[:, pair, ts0:ts0 + TCH], in_=qp[:])
                    kp = qk_ps.tile([128, TCH], F32)
                    for h0, h1 in halves:
                        for kc in range(KC):
                            nc.tensor.matmul(kp[:, h0:h1], wk_sb[:, kc, psl], xts[kc][:, h0:h1],
                                             start=(kc == 0), stop=(kc == KC - 1))
                    nc.vector.tensor_copy(out=kt_sb[:, pair, ts0:ts0 + TCH], in_=kp[:])
                for tsub in range(TCH // KB):
                    vp = v_ps.tile([128, GE], F32)
                    for kc in range(KC):
                        nc.tensor.matmul(vp[:], xts[kc][:, tsub * KB:(tsub + 1) * KB], wv_sb[:, kc, :],
                                         start=(kc == 0), stop=(kc == KC - 1))
                    tb = tch * (TCH // KB) + tsub
                    nc.vector.tensor_copy(
                        out=v_sb[:, tb, :].rearrange("p (h c) -> p h c", c=D + 1)[:, :, 0:D],
                        in_=vp.rearrange("p (h c) -> p h c", c=D),
                    )

            proj_ctx.close()

            # --- phase 2+3: attention per q-block, then its output projection ---
            attn_ctx = ExitStack()
            s_ps = attn_ctx.enter_context(tc.tile_pool(name="s_ps", bufs=o["s_bufs"], space="PSUM"))
            o_ps = attn_ctx.enter_context(tc.tile_pool(name="o_ps", bufs=o["o_bufs"], space="PSUM"))
            if o["y_in_s"]:
                y_ps, y_tag = s_ps, "s"
            elif o.get("y_in_o", False):
                y_ps, y_tag = o_ps, "o"
            else:
                y_ps = attn_ctx.enter_context(
                    tc.tile_pool(name="y_ps", bufs=o.get("y_bufs", 1), space="PSUM"))
                y_tag = "y"

            def slot(hb):
                return slice(hb * (D + 1), (hb + 1) * (D + 1))

            def tri_eng(pt, r):
                eng = nc.gpsimd if o.get("tri_gpsimd", False) else nc.vector
                eng.tensor_mul(pt[:, r:r + KB], pt[:, r:r + KB], tri_sb[:])

            def normalize(o_p, onorm, h, splits=1):
                # reciprocal of the l row (PSUM partition 64 -> SBUF partition
                # 0; DVE handles the base shift), gpsimd-broadcast across 64
                # partitions, then normalize straight out of PSUM into the
                # pair-stacked onorm tile (head B writes partitions 64:128).
                # splits>1 chops the chain along q so downstream Y matmuls
                # start on the first q-tile sooner (used for the last q-block).
                w = QB // splits
                for s in range(splits):
                    qs = slice(s * w, (s + 1) * w)
                    strip = lpool.tile([1, w], F32, tag="strip")
                    if o.get("recip_fast", True):
                        nc.vector.reciprocal_approx_fast(out=strip[:], in_=o_p[D:D + 1, qs])
                    else:
                        nc.vector.reciprocal(out=strip[:], in_=o_p[D:D + 1, qs])
                    lb = lpool.tile([D, w], F32, tag="lb")
                    nc.gpsimd.partition_broadcast(lb[:], strip[:])
                    nc.vector.tensor_mul(onorm[h * D:(h + 1) * D, qs], o_p[0:D, qs], lb[:])

            for qb in o.get("qb_order", list(range(NQB))):
                q0 = qb * QB
                nk = (q0 + QB) // KB          # kblocks 0..nk-1; last 4 are diagonal
                nfull = nk - 4
                onorms = []
                for pair in range(2):
                    onorm = onpool.tile([128, QB], F32R)
                    heads = [0, 1] if o["interleave"] else None
                    if o["interleave"]:
                        o_ps_t = [o_ps.tile([D + 1, QB], F32, tag="o", name="o_t") for _ in range(2)]
                        qr = [qt_sb[h * D:(h + 1) * D, pair, q0:q0 + QB] for h in range(2)]
                        # full k-blocks two at a time; S matmuls for the two
                        # heads adjacent (disjoint PE row groups -> HW overlap)
                        for j2 in range(0, nfull, 2):
                            sps = [s_ps.tile([128, 2 * QB], F32, tag="s", name="s_t") for _ in range(2)]
                            for jj in range(2):
                                j = j2 + jj
                                for h in range(2):
                                    nc.tensor.matmul(
                                        sps[h][:, jj * QB:(jj + 1) * QB],
                                        kt_sb[h * D:(h + 1) * D, pair, j * KB:(j + 1) * KB],
                                        qr[h], start=True, stop=True)
                            pts = []
                            for h in range(2):
                                pt = ppool.tile([128, 2 * QB], F32R, tag="p")
                                nc.scalar.activation(out=pt[:], in_=sps[h][:], func=EXP, scale=SCALE)
                                pts.append(pt)
                            for jj in range(2):
                                j = j2 + jj
                                for h in range(2):
                                    nc.tensor.matmul(
                                        o_ps_t[h][:], v_sb[:, j, slot(pair * 2 + h)],
                                        pts[h][:, jj * QB:(jj + 1) * QB],
                                        start=(j == 0), stop=False)
                        for j in range(nfull, nk):
                            r = (j - nfull) * KB
                            sps = [s_ps.tile([128, 2 * QB], F32, tag="s", name="s_t") for _ in range(2)]
                            for h in range(2):
                                nc.tensor.matmul(
                                    sps[h][:, 0:QB],
                                    kt_sb[h * D:(h + 1) * D, pair, j * KB:(j + 1) * KB],
                                    qr[h], start=True, stop=True)
                            for h in range(2):
                                pt = ppool.tile([128, 2 * QB], F32R, tag="p")
                                nc.scalar.activation(out=pt[:, r:QB], in_=sps[h][:, r:QB], func=EXP, scale=SCALE)
                                tri_eng(pt, r)
                                nc.tensor.matmul(
                                    o_ps_t[h][:, r:QB], v_sb[:, j, slot(pair * 2 + h)],
                                    pt[:, r:QB], start=(j == 0), stop=(j == nk - 1))
                        for h in range(2):
                            normalize(o_ps_t[h], onorm, h)
                    else:
                        for h in range(2):
                            hb = pair * 2 + h
                            bsl = slice(h * D, h * D + D)
                            o_p = o_ps.tile([D + 1, QB], F32)
                            qrhs = qt_sb[bsl, pair, q0:q0 + QB]
                            for j2 in range(0, nfull, 2):
                                sp = s_ps.tile([128, 2 * QB], F32, tag="s")
                                for jj in range(2):
                                    j = j2 + jj
                                    nc.tensor.matmul(sp[:, jj * QB:(jj + 1) * QB],
                                                     kt_sb[bsl, pair, j * KB:(j + 1) * KB],
                                                     qrhs, start=True, stop=True)
                                pt = ppool.tile([128, 2 * QB], F32R, tag="p")
                                nc.scalar.activation(out=pt[:], in_=sp[:], func=EXP, scale=SCALE)
                                for jj in range(2):
                                    j = j2 + jj
                                    nc.tensor.matmul(o_p[:], v_sb[:, j, slot(hb)],
                                                     pt[:, jj * QB:(jj + 1) * QB],
                                                     start=(j == 0), stop=False)
                            for j2 in range(nfull, nk, 2):
                                # two diagonal k-blocks share one 2-bank psum
                                # tile and a single exp over [r0 : QB+r1+KB]
                                # (the [QB : QB+r1) strip is unread garbage)
                                r0 = (j2 - nfull) * KB
                                r1 = r0 + KB
                                sp = s_ps.tile([128, 2 * QB], F32, tag="s")
                                for jj in range(2):
                                    j = j2 + jj
                                    nc.tensor.matmul(sp[:, jj * QB:(jj + 1) * QB],
                                                     kt_sb[bsl, pair, j * KB:(j + 1) * KB],
                                                     qrhs, start=True, stop=True)
                                pt = ppool.tile([128, 2 * QB], F32R, tag="p")
                                nc.scalar.activation(out=pt[:, r0:2 * QB], in_=sp[:, r0:2 * QB],
                                                     func=EXP, scale=SCALE)
                                for jj, r in ((0, r0), (1, r1)):
                                    j = j2 + jj
                                    base_c = jj * QB
                                    nc.vector.tensor_mul(pt[:, base_c + r:base_c + r + KB],
                                                         pt[:, base_c + r:base_c + r + KB], tri_sb[:])
                                    nc.tensor.matmul(o_p[:, r:QB], v_sb[:, j, slot(hb)],
                                                     pt[:, base_c + r:base_c + QB],
                                                     start=(j == 0), stop=(j == nk - 1))
                            normalize(o_p, onorm, h,
                                      splits=(o.get("norm_splits_last", 1) if qb == NQB - 1 else 1))
                    onorms.append(onorm)
                # output projection for this q-block
                for qt in range(QB // 128):
                    if o.get("y_split", False):
                        yt = ypool.tile([128, E], F32)
                        for nh in range(2):
                            if o.get("y_in_o", False):
                                yp = o_ps.tile([128, 512], F32, tag="o" if o["interleave"] else "o_p", name="yp")
                            elif o.get("y_last_in_s", False) and qb == NQB - 1:
                                yp = s_ps.tile([128, 512], F32, tag="s", name="yp")
                            else:
                                yp = y_ps.tile([128, 512], F32, tag=y_tag, name="yp")
                            for pair in range(2):
                                nc.tensor.matmul(yp[:],
                                                 onorms[pair][:, qt * 128:(qt + 1) * 128],
                                                 wp_sb[:, pair, nh * 512:(nh + 1) * 512],
                                                 start=(pair == 0), stop=(pair == 1))
                            if o.get("y_copy_act", False) and nh == 1:
                                nc.scalar.activation(out=yt[:, nh * 512:(nh + 1) * 512], in_=yp[:],
                                                     func=mybir.ActivationFunctionType.Copy)
                            else:
                                nc.vector.tensor_copy(out=yt[:, nh * 512:(nh + 1) * 512], in_=yp[:])
                        nc.sync.dma_start(out=y_d[q0 + qt * 128:q0 + (qt + 1) * 128, :], in_=yt[:])
                    else:
                        yp = y_ps.tile([128, E], F32, tag=y_tag)
                        for nh in range(2):
                            for pair in range(2):
                                nc.tensor.matmul(yp[:, nh * 512:(nh + 1) * 512],
                                                 onorms[pair][:, qt * 128:(qt + 1) * 128],
                                                 wp_sb[:, pair, nh * 512:(nh + 1) * 512],
                                                 start=(pair == 0), stop=(pair == 1))
                        yt = ypool.tile([128, E], F32)
                        nc.vector.tensor_copy(out=yt[:], in_=yp[:])
                        nc.sync.dma_start(out=y_d[q0 + qt * 128:q0 + (qt + 1) * 128, :], in_=yt[:])
            attn_ctx.close()

    nc.compile()
    return nc


_NC = {}


def _get_program(opts=None):
    key = tuple(sorted((opts or {}).items()))
    if key not in _NC:
        _NC[key] = build_program(opts)
    return _NC[key]


def _make_in_maps(x, Wq, Wk, Wv, Wp):
    x = np.asarray(x, dtype=np.float32)
    wqt = np.ascontiguousarray(np.asarray(Wq, np.float32).T)
    wkt = np.ascontiguousarray(np.asarray(Wk, np.float32).T)
    wvt = np.ascontiguousarray(np.asarray(Wv, np.float32).T)
    wpt = np.ascontiguousarray(np.asarray(Wp, np.float32).T)
    tri = (np.arange(KB)[:, None] <= np.arange(KB)[None, :]).astype(np.float32)
    ones = np.ones((128, (T // KB) * GH), np.float32)
    in_maps = []
    for c in range(N_CORES):
        b, hg = c // 4, c % 4
        in_maps.append({
            "xt": np.ascontiguousarray(x[b].T),
            "wqt": np.ascontiguousarray(wqt[:, hg * GE:(hg + 1) * GE]),
            "wkt": np.ascontiguousarray(wkt[:, hg * GE:(hg + 1) * GE]),
            "wvt": np.ascontiguousarray(wvt[:, hg * GE:(hg + 1) * GE]),
            "wpt": np.ascontiguousarray(wpt[hg * GE:(hg + 1) * GE, :]),
            "tri": tri,
            "ones": ones,
        })
    return in_maps


def run_cores(x, Wq, Wk, Wv, Wp, bp, **spmd_kwargs):
    """Run the 8-core program; returns (y_full, BassKernelResults)."""
    nc = _get_program()
    in_maps = _make_in_maps(x, Wq, Wk, Wv, Wp)
    res = run_bass_kernel_spmd(nc, in_maps, list(range(N_CORES)), **spmd_kwargs)
    parts = [res.results[c]["y"] for c in range(N_CORES)]
    y = np.empty((B, T, E), np.float32)
    for b in range(B):
        acc = parts[4 * b].astype(np.float32)
        for hg in range(1, 4):
            acc = acc + parts[4 * b + hg]
        y[b] = acc
    y += np.asarray(bp, np.float32)[None, None, :]
    return y, res


def kernel(x, Wq, Wk, Wv, Wp, bp):
    y, _ = run_cores(x, Wq, Wk, Wv, Wp, bp)
    return y


def bench(x, Wq, Wk, Wv, Wp, bp, iters=12):
    """Time repeated on-device executions of the compiled program.

    Returns (y_full, list_of_call_seconds). Builds the sharded jit once;
    inputs are device-resident; fresh donated zero outputs are staged
    outside the timed region each iteration.
    """
    import time

    import jax
    import numpy as np_
    from jax.experimental.shard_map import shard_map
    from jax.sharding import Mesh, NamedSharding, PartitionSpec

    from concourse import bass2jax, mybir as mb

    nc = _get_program()
    in_maps = _make_in_maps(x, Wq, Wk, Wv, Wp)
    n_cores = N_CORES
    bass2jax.install_neuronx_cc_hook()

    partition_name = nc.partition_id_tensor.name if nc.partition_id_tensor else None
    in_names, out_names, out_avals, zero_outs = [], [], [], []
    for alloc in nc.m.functions[0].allocations:
        if not isinstance(alloc, mb.MemoryLocationSet):
            continue
        name = alloc.memorylocations[0].name
        if alloc.kind == "ExternalInput":
            if name != partition_name:
                in_names.append(name)
        elif alloc.kind == "ExternalOutput":
            out_names.append(name)
            shape = tuple(alloc.tensor_shape)
            dtype = mb.dt.np(alloc.dtype)
            out_avals.append(jax.core.ShapedArray(shape, dtype))
            zero_outs.append(np_.zeros(shape, dtype))
    n_params = len(in_names)
    all_in_names = in_names + out_names
    if partition_name is not None:
        all_in_names = all_in_names + [partition_name]

    def _body(*args):
        operands = list(args)
        if partition_name is not None:
            operands.append(bass2jax.partition_id_tensor())
        outs = bass2jax._bass_exec_p.bind(
            *operands,
            out_avals=tuple(out_avals),
            in_names=tuple(all_in_names),
            out_names=tuple(out_names),
            lowering_input_output_aliases=(),
            sim_require_finite=True,
            sim_require_nnan=True,
            nc=nc,
        )
        return tuple(outs)

    devices = jax.devices()[:n_cores]
    mesh = Mesh(np_.asarray(devices), ("core",))
    donate = tuple(range(n_params, n_params + len(out_names)))
    sharded = jax.jit(
        shard_map(_body, mesh=mesh,
                  in_specs=(PartitionSpec("core"),) * (n_params + len(out_names)),
                  out_specs=(PartitionSpec("core"),) * len(out_names),
                  check_rep=False),
        donate_argnums=donate, keep_unused=True,
    )
    sh = NamedSharding(mesh, PartitionSpec("core"))
    concat_in = [
        jax.device_put(
            np_.concatenate([np_.asarray(in_maps[c][nm]) for c in range(n_cores)], axis=0), sh)
        for nm in in_names
    ]
    zeros_np = [np_.zeros((n_cores * z.shape[0], *z.shape[1:]), z.dtype) for z in zero_outs]

    times = []
    out_arrs = None
    for it in range(iters):
        dz = [jax.device_put(z, sh) for z in zeros_np]
        jax.block_until_ready(dz)
        t0 = time.perf_counter()
        out_arrs = sharded(*concat_in, *dz)
        jax.block_until_ready(out_arrs)
        times.append(time.perf_counter() - t0)

    parts = [
        np_.asarray(out_arrs[i]).reshape(n_cores, *out_avals[i].shape)
        for i, nm in enumerate(out_names)
    ]
    yi = out_names.index("y")
    y = np_.empty((B, T, E), np_.float32)
    for b in range(B):
        acc = parts[yi][4 * b].astype(np_.float32)
        for hg in range(1, 4):
            acc = acc + parts[yi][4 * b + hg]
        y[b] = acc
    y += np_.asarray(bp, np_.float32)[None, None, :]
    return y, times
